# revision 2
# baseline (speedup 1.0000x reference)
"""All-pole IIR filter (order 16) on 8 Trainium2 NeuronCores.

Math: y[t] = x[t] - sum_{k=1..16} a_k y[t-k]  (per (b,c) lane, zero init).

The impulse response h decays fast (|poles| <~ 0.91): truncate to 128
taps. h[0]=1, so split y = x + c with c = g*x, g = h[1:128]; the device
computes only the correction c with all streams fp8e4m3 (~6.6e-3 rel
err vs the 2e-2 gate) and the host adds the exact f32 x back.

Device blocking, Q=128: output chunk t needs x chunks t-1 and t. Per
lane TWO normal-mode fp8 matmuls accumulate into one PSUM bank:
    ps  = W1^T @ x[cols 1:513]   (within-chunk taps,  start=True)
    ps += W0^T @ x[cols 0:512]   (cross-chunk taps,   stop=True)
with W1[q,i] = g[i-q] (i>q), W0[q,i] = g[128+i-q] (q>i), and the per-
lane x tile [128, 513] holding a leading zero chunk then chunk u at
col u+1. Normal mode streams 1 col/cycle at 2.4 GHz (fp8 = bf16
speed; 512-col matmul ~215 ns warm) with compiler-auto FWL weight
loads. Output lands PSUM[i, t] in natural order.

HW findings that shaped this (measured via ntff traces on trn2):
  * DoubleRow with an overlapping k-tile moving AP (stride 1 elem)
    computes correctly once but HANGS the PE when two such matmuls run
    back-to-back (both streams hit the same 16-B SBUF line); a 16-lane
    interleaved layout (k-tile stride 16 B) is correct but the
    scattered 1-of-16-byte reads run ~3x slower. DR with adjacent
    panels needs 1.5x weight bytes and its 256-row LDWEIGHTS (~160 ns)
    eats the column-rate win. Hence plain normal mode.
  * A second concurrent DMA ring (scalar or gpsimd queue) drops the
    whole chip ~1.2x (MM 218->260 ns, ACT 1114->1336) -- power
    throttle. Everything rides the single sync-queue ring: loads in
    consumption order, then stores.
  * The HAM activity governor runs the PE at 1.2 GHz until a full
    ~3.4 us window is busy. Real matmuls cannot start before lane 0's
    DMA lands (~12 us in), so 18 dummy matmuls on a scratch tile warm
    the PE during the idle 7.5-11.5 us stretch; the real stream then
    runs at 2.4 GHz from its first matmul.
  * DMA efficiency needs >= 1 KiB contiguous per partition row
    (single-lane 512-B rows run ~4x slower), and each DMA trigger
    instruction costs ~0.65 us on its engine -- so lanes load in
    groups of 4/8/16 and store in groups of 4.

Per core (32 lanes): 64 LDWEIGHTS+MATMUL, 16 PSUM->fp8 pair-casts
alternating ACT/DVE (the last pair per-lane on both engines to cut
the drain tail), DMA x 2.0 + w 1.0 + c 2.0 MiB. HW exec ~34 us
(baseline corrf8 DoubleRow kernel: ~37 us), ~26 us of which is the
compute window and ~8 us fixed NEFF preamble + semaphore-file-reset
epilogue.
"""

import numpy as np
from contextlib import ExitStack

B, C, T = 32, 8, 65536
L = B * C              # 256 independent lanes
NCORES = 8
LPC = L // NCORES      # 32 lanes per core
Q = 128                # chunk length
NCH = T // Q           # 512 chunks per lane
XC = NCH + 1           # x cols per lane incl leading zero chunk
G = 4                  # lanes per store group
WG = [(0, 4), (4, 8), (8, 16), (16, 32)]
XG = [(0, 4), (4, 8), (8, 16), (16, 24), (24, 32)]

_cache = {}


def _build():
    import concourse.tile as tile
    from concourse import bacc, mybir

    F32 = mybir.dt.float32
    F8 = mybir.dt.float8e4
    nc = bacc.Bacc("TRN2", target_bir_lowering=False, debug=False)

    # Per-core DRAM layouts (lane dim contiguous within partition rows):
    #   xq: [Q, LPC, XC]    xq[q, l, 1+u] = x_l[128 u + q], col 0 zeros
    #   w2: [Q, LPC, 2, Q]  per-lane stationary slots [W0, W1]
    #   c:  [Q, LPC, NCH]   c[i, l, t] = correction for sample 128 t + i
    xq_d = nc.dram_tensor("xq", [Q, LPC, XC], F8, kind="ExternalInput")
    w2_d = nc.dram_tensor("w2", [Q, LPC, 2, Q], F8, kind="ExternalInput")
    c_d = nc.dram_tensor("c", [Q, LPC, NCH], F8, kind="ExternalOutput")

    with tile.TileContext(nc) as tc:
        with ExitStack() as ctx:
            wpool = ctx.enter_context(tc.tile_pool(name="w", bufs=1))
            xpool = ctx.enter_context(tc.tile_pool(name="x", bufs=1))
            cpool = ctx.enter_context(tc.tile_pool(name="c", bufs=1))
            pspool = ctx.enter_context(
                tc.tile_pool(name="ps", bufs=3, space="PSUM")
            )
            scrpool = ctx.enter_context(
                tc.tile_pool(name="scr", bufs=1, space="PSUM")
            )

            # PE warm-up: the HAM throttle caps the PE at 1.2 GHz until
            # a full ~3.4 us activity window is busy; real matmuls can't
            # start until lane 0's DMA lands (~12 us), so fill the idle
            # 7.5-11.6 us with dummy matmuls on a scratch tile. Runs on
            # the otherwise-idle tensor engine; with all DMA on the one
            # sync ring this does not trip the chip-level power throttle
            # (a second concurrent DMA ring does: everything drops ~1.2x).
            scr = wpool.tile([Q, 256], F8, tag="scr", name="scr")
            nc.gpsimd.memset(scr[:], 0)
            pscr = scrpool.tile([Q, 256], F32, tag="pscr", name="pscr")
            for _ in range(18):
                nc.tensor.matmul(pscr[:], scr[:, 0:Q], scr[:, :],
                                 start=True, stop=True)

            # All tiles resident (~40 KiB/partition total).
            wt = [wpool.tile([Q, hi - lo, 2, Q], F8, tag=f"w{k}",
                             name=f"w{k}") for k, (lo, hi) in enumerate(WG)]
            xt = [xpool.tile([Q, hi - lo, XC], F8, tag=f"x{k}",
                             name=f"x{k}") for k, (lo, hi) in enumerate(XG)]

            # Loads on the sync queue in consumption order; small groups
            # first so lane 0's operands land ~1 us after queue start.
            order = [("w", 0), ("x", 0), ("w", 1), ("x", 1), ("w", 2),
                     ("x", 2), ("w", 3), ("x", 3), ("x", 4)]
            for kind, k in order:
                if kind == "w":
                    lo, hi = WG[k]
                    nc.sync.dma_start(wt[k][:], w2_d.ap()[:, lo:hi, :, :])
                else:
                    lo, hi = XG[k]
                    nc.sync.dma_start(xt[k][:], xq_d.ap()[:, lo:hi, :])

            def lane_slices(lane):
                wk = next(k for k, (lo, hi) in enumerate(WG) if lane < hi)
                xk = next(k for k, (lo, hi) in enumerate(XG) if lane < hi)
                return (wt[wk], lane - WG[wk][0]), (xt[xk], lane - XG[xk][0])

            NGRP = LPC // G
            for g in range(NGRP):
                ct = cpool.tile([Q, G, NCH], F8, tag=f"c{g}", name=f"c{g}")
                for p in range(G // 2):
                    ps = pspool.tile([Q, 2, NCH], F32, tag="ps", name="ps_t")
                    for e in range(2):
                        lane = g * G + 2 * p + e
                        (wtile, wj), (xtile, xj) = lane_slices(lane)
                        nc.tensor.matmul(
                            ps[:, e, :], wtile[:, wj, 1, :],
                            xtile[:, xj, 1:XC],
                            start=True, stop=False,
                        )
                        nc.tensor.matmul(
                            ps[:, e, :], wtile[:, wj, 0, :],
                            xtile[:, xj, 0:NCH],
                            start=False, stop=True,
                        )
                    if g == NGRP - 1 and p == G // 2 - 1:
                        # final pair: per-lane casts, ACT || DVE in
                        # parallel, so the last cast tail is ~0.7 us
                        nc.scalar.copy(ct[:, 2 * p : 2 * p + 1, :],
                                       ps[:, 0:1, :])
                        nc.vector.tensor_copy(
                            ct[:, 2 * p + 1 : 2 * p + 2, :], ps[:, 1:2, :]
                        )
                    else:
                        dst = ct[:, 2 * p : 2 * p + 2, :]
                        if (2 * g + p) % 2 == 0:
                            nc.scalar.copy(dst, ps[:])
                        else:
                            nc.vector.tensor_copy(dst, ps[:])
                if g == NGRP - 1:
                    # split the last store so the drain tail is shorter
                    h = G // 2
                    nc.sync.dma_start(
                        c_d.ap()[:, g * G : g * G + h, :], ct[:, 0:h, :]
                    )
                    nc.sync.dma_start(
                        c_d.ap()[:, g * G + h : (g + 1) * G, :], ct[:, h:G, :]
                    )
                else:
                    sl = slice(g * G, (g + 1) * G)
                    nc.sync.dma_start(c_d.ap()[:, sl, :], ct[:])

    nc.compile()
    return nc


def _get_bass():
    if "nc" not in _cache:
        _cache["nc"] = _build()
    return _cache["nc"]


def _impulse_response(a: np.ndarray, ktaps: int) -> np.ndarray:
    """h[l, n] for n in [0, ktaps), float64 recurrence."""
    an = (a.astype(np.float64) / a[..., 0:1].astype(np.float64)).reshape(
        L, a.shape[-1]
    )
    h = np.zeros((L, ktaps), np.float64)
    h[:, 0] = 1.0
    p = a.shape[-1] - 1
    for n in range(1, ktaps):
        k = np.arange(1, min(n, p) + 1)
        h[:, n] = -np.einsum("lk,lk->l", an[:, k], h[:, n - k])
    return h


def _pack(x: np.ndarray, a: np.ndarray):
    import ml_dtypes

    F8 = ml_dtypes.float8_e4m3

    g = _impulse_response(a, Q).astype(np.float32)  # [L, 128]
    g[:, 0] = 0.0  # tap 0 handled exactly on host (y = x + c)

    qi = np.arange(Q)
    D = qi[None, :] - qi[:, None]  # D[q, i] = i - q
    W1 = np.where(D >= 1, g[:, np.clip(D, 0, Q - 1)], 0.0)
    W0 = np.where(D <= -1, g[:, np.clip(D + Q, 0, Q - 1)], 0.0)
    w2 = np.stack([W0, W1], axis=2)  # [L, q, 2, i]
    w2 = np.ascontiguousarray(w2.transpose(1, 0, 2, 3)).astype(F8)

    x3 = x.reshape(L, NCH, Q)
    xq = np.zeros((Q, L, XC), F8)
    xq[:, :, 1:] = x3.transpose(2, 0, 1).astype(F8)

    in_maps = []
    for core in range(NCORES):
        sl = slice(core * LPC, (core + 1) * LPC)
        in_maps.append(
            {
                "xq": np.ascontiguousarray(xq[:, sl, :]),
                "w2": np.ascontiguousarray(w2[:, sl, :, :]),
            }
        )
    return in_maps


def kernel(x: np.ndarray, a: np.ndarray) -> np.ndarray:
    from concourse import bass_utils

    x = np.ascontiguousarray(x, dtype=np.float32)
    a = np.ascontiguousarray(a, dtype=np.float32)
    in_maps = _pack(x, a)

    nc = _get_bass()
    res = bass_utils.run_bass_kernel_spmd(
        nc,
        in_maps,
        core_ids=list(range(NCORES)),
        trace=bool(_cache.get("trace", False)),
        trace_cores=_cache.get("trace_cores"),
    )
    _cache["last_results"] = res

    y = np.empty((L, T), np.float32)
    for core in range(NCORES):
        c = res.results[core]["c"].astype(np.float32)  # [i, lane, t]
        sl = slice(core * LPC, (core + 1) * LPC)
        y[sl] = c.transpose(1, 2, 0).reshape(LPC, T)  # [lane, t, i]
    y += x.reshape(L, T)
    return y.reshape(B, C, T)


# revision 3
# speedup vs baseline: 1.1532x; 1.1532x over previous
"""All-pole IIR filter (order 16) on 8 Trainium2 NeuronCores.

Math: y[t] = x[t] - sum_{k=1..16} a_k y[t-k]  (per (b,c) lane, zero init).

The impulse response h decays fast (|poles| <~ 0.91): truncate to 128
taps. h[0]=1, so split y = x + c with c = g*x, g = h[1:128]; the device
computes only the correction c with all streams fp8e4m3 (~6.6e-3 rel
err vs the 2e-2 gate) and the host adds the exact f32 x back.

Device blocking, Q=128: output chunk t needs x chunks t-1 and t. Per
lane TWO normal-mode fp8 matmuls accumulate into one PSUM bank:
    ps  = W1^T @ x[cols 1:513]   (within-chunk taps,  start=True)
    ps += W0^T @ x[cols 0:512]   (cross-chunk taps,   stop=True)
with W1[q,i] = g[i-q] (i>q), W0[q,i] = g[128+i-q] (q>i), and the per-
lane x tile [128, 513] holding a leading zero chunk then chunk u at
col u+1. Normal mode streams 1 col/cycle at 2.4 GHz (fp8 = bf16
speed; 512-col matmul ~215 ns warm) with compiler-auto FWL weight
loads. Output lands PSUM[i, t] in natural order.

HW findings that shaped this (measured via ntff traces on trn2):
  * DoubleRow with an overlapping k-tile moving AP (stride 1 elem)
    computes correctly once but HANGS the PE when two such matmuls run
    back-to-back (both streams hit the same 16-B SBUF line); a 16-lane
    interleaved layout (k-tile stride 16 B) is correct but the
    scattered 1-of-16-byte reads run ~3x slower. DR with adjacent
    panels needs 1.5x weight bytes and its 256-row LDWEIGHTS (~160 ns)
    eats the column-rate win. Hence plain normal mode.
  * A second concurrent DMA ring (scalar or gpsimd queue) drops the
    whole chip ~1.2x (MM 218->260 ns, ACT 1114->1336) -- power
    throttle. Everything rides the single sync-queue ring: loads in
    consumption order, then stores.
  * The HAM activity governor runs the PE at 1.2 GHz until a full
    ~3.4 us window is busy. Real matmuls cannot start before lane 0's
    DMA lands (~12 us in), so 18 dummy matmuls on a scratch tile warm
    the PE during the idle 7.5-11.5 us stretch; the real stream then
    runs at 2.4 GHz from its first matmul.
  * DMA efficiency needs >= 1 KiB contiguous per partition row
    (single-lane 512-B rows run ~4x slower), and each DMA trigger
    instruction costs ~0.65 us on its engine -- so lanes load in
    groups of 4/8/16 and store in groups of 4.

Per core (32 lanes): 64 LDWEIGHTS+MATMUL, 16 PSUM->fp8 pair-casts
alternating ACT/DVE (the last pair per-lane on both engines to cut
the drain tail), DMA x 2.0 + w 1.0 + c 2.0 MiB. HW exec ~34 us
(baseline corrf8 DoubleRow kernel: ~37 us), ~26 us of which is the
compute window and ~8 us fixed NEFF preamble + semaphore-file-reset
epilogue.
"""

import numpy as np
from contextlib import ExitStack

B, C, T = 32, 8, 65536
L = B * C              # 256 independent lanes
NCORES = 8
LPC = L // NCORES      # 32 lanes per core
Q = 128                # chunk length
NCH = T // Q           # 512 chunks per lane
XC = NCH + 1           # x cols per lane incl leading zero chunk
G = 4                  # lanes per store group
WG = [(0, 4), (4, 8), (8, 16), (16, 24), (24, 32)]
XG = [(0, 4), (4, 8), (8, 16), (16, 24), (24, 32)]

_cache = {}


def _build():
    import concourse.tile as tile
    from concourse import bacc, mybir

    F32 = mybir.dt.float32
    F8 = mybir.dt.float8e4
    nc = bacc.Bacc("TRN2", target_bir_lowering=False, debug=False)

    # Per-core DRAM layouts (lane dim contiguous within partition rows):
    #   xq: [Q, LPC, XC]    xq[q, l, 1+u] = x_l[128 u + q], col 0 zeros
    #   w2: [Q, LPC, 2, Q]  per-lane stationary slots [W0, W1]
    #   c:  [Q, LPC, NCH]   c[i, l, t] = correction for sample 128 t + i
    xq_d = nc.dram_tensor("xq", [Q, LPC, XC], F8, kind="ExternalInput")
    w2_d = nc.dram_tensor("w2", [Q, LPC, 2, Q], F8, kind="ExternalInput")
    c_d = nc.dram_tensor("c", [Q, LPC, NCH], F8, kind="ExternalOutput")

    with tile.TileContext(nc) as tc:
        with ExitStack() as ctx:
            wpool = ctx.enter_context(tc.tile_pool(name="w", bufs=1))
            xpool = ctx.enter_context(tc.tile_pool(name="x", bufs=1))
            cpool = ctx.enter_context(tc.tile_pool(name="c", bufs=1))
            pspool = ctx.enter_context(
                tc.tile_pool(name="ps", bufs=3, space="PSUM")
            )
            scrpool = ctx.enter_context(
                tc.tile_pool(name="scr", bufs=1, space="PSUM")
            )

            # PE warm-up: the HAM throttle caps the PE at 1.2 GHz until
            # a full ~3.4 us activity window is busy; real matmuls can't
            # start until lane 0's DMA lands (~12 us), so fill the idle
            # 7.5-11.6 us with dummy matmuls on a scratch tile. Runs on
            # the otherwise-idle tensor engine; with all DMA on the one
            # sync ring this does not trip the chip-level power throttle
            # (a second concurrent DMA ring does: everything drops ~1.2x).
            scr = wpool.tile([Q, 256], F8, tag="scr", name="scr")
            nc.gpsimd.memset(scr[:], 0)
            pscr = scrpool.tile([Q, 256], F32, tag="pscr", name="pscr")
            for _ in range(14):
                nc.tensor.matmul(pscr[:], scr[:, 0:Q], scr[:, :],
                                 start=True, stop=True)

            # All tiles resident (~40 KiB/partition total).
            wt = [wpool.tile([Q, hi - lo, 2, Q], F8, tag=f"w{k}",
                             name=f"w{k}") for k, (lo, hi) in enumerate(WG)]
            xt = [xpool.tile([Q, hi - lo, XC], F8, tag=f"x{k}",
                             name=f"x{k}") for k, (lo, hi) in enumerate(XG)]

            # Loads on the sync queue in consumption order; small groups
            # first so lane 0's operands land ~1 us after queue start.
            order = [("w", 0), ("x", 0), ("w", 1), ("x", 1), ("x", 2),
                     ("w", 2), ("x", 3), ("w", 3), ("x", 4), ("w", 4)]
            for kind, k in order:
                if kind == "w":
                    lo, hi = WG[k]
                    nc.sync.dma_start(wt[k][:], w2_d.ap()[:, lo:hi, :, :])
                else:
                    lo, hi = XG[k]
                    nc.sync.dma_start(xt[k][:], xq_d.ap()[:, lo:hi, :])

            def lane_slices(lane):
                wk = next(k for k, (lo, hi) in enumerate(WG) if lane < hi)
                xk = next(k for k, (lo, hi) in enumerate(XG) if lane < hi)
                return (wt[wk], lane - WG[wk][0]), (xt[xk], lane - XG[xk][0])

            NGRP = LPC // G
            for g in range(NGRP):
                ct = cpool.tile([Q, G, NCH], F8, tag=f"c{g}", name=f"c{g}")
                for p in range(G // 2):
                    ps = pspool.tile([Q, 2, NCH], F32, tag="ps", name="ps_t")
                    for e in range(2):
                        lane = g * G + 2 * p + e
                        (wtile, wj), (xtile, xj) = lane_slices(lane)
                        nc.tensor.matmul(
                            ps[:, e, :], wtile[:, wj, 1, :],
                            xtile[:, xj, 1:XC],
                            start=True, stop=False,
                        )
                        nc.tensor.matmul(
                            ps[:, e, :], wtile[:, wj, 0, :],
                            xtile[:, xj, 0:NCH],
                            start=False, stop=True,
                        )
                    if g == NGRP - 1 and p == G // 2 - 1:
                        # final pair: per-lane casts, ACT || DVE in
                        # parallel, so the last cast tail is ~0.7 us
                        nc.scalar.copy(ct[:, 2 * p : 2 * p + 1, :],
                                       ps[:, 0:1, :])
                        nc.vector.tensor_copy(
                            ct[:, 2 * p + 1 : 2 * p + 2, :], ps[:, 1:2, :]
                        )
                    else:
                        dst = ct[:, 2 * p : 2 * p + 2, :]
                        if (2 * g + p) % 2 == 0:
                            nc.scalar.copy(dst, ps[:])
                        else:
                            nc.vector.tensor_copy(dst, ps[:])
                if g == NGRP - 1:
                    # split the last store so the drain tail is shorter
                    h = G // 2
                    nc.sync.dma_start(
                        c_d.ap()[:, g * G : g * G + h, :], ct[:, 0:h, :]
                    )
                    nc.sync.dma_start(
                        c_d.ap()[:, g * G + h : (g + 1) * G, :], ct[:, h:G, :]
                    )
                else:
                    sl = slice(g * G, (g + 1) * G)
                    nc.sync.dma_start(c_d.ap()[:, sl, :], ct[:])

    nc.compile()
    return nc


def _get_bass():
    if "nc" not in _cache:
        _cache["nc"] = _build()
    return _cache["nc"]


def _impulse_response(a: np.ndarray, ktaps: int) -> np.ndarray:
    """h[l, n] for n in [0, ktaps), float64 recurrence."""
    an = (a.astype(np.float64) / a[..., 0:1].astype(np.float64)).reshape(
        L, a.shape[-1]
    )
    h = np.zeros((L, ktaps), np.float64)
    h[:, 0] = 1.0
    p = a.shape[-1] - 1
    for n in range(1, ktaps):
        k = np.arange(1, min(n, p) + 1)
        h[:, n] = -np.einsum("lk,lk->l", an[:, k], h[:, n - k])
    return h


def _pack(x: np.ndarray, a: np.ndarray):
    import ml_dtypes

    F8 = ml_dtypes.float8_e4m3

    g = _impulse_response(a, Q).astype(np.float32)  # [L, 128]
    g[:, 0] = 0.0  # tap 0 handled exactly on host (y = x + c)

    qi = np.arange(Q)
    D = qi[None, :] - qi[:, None]  # D[q, i] = i - q
    W1 = np.where(D >= 1, g[:, np.clip(D, 0, Q - 1)], 0.0)
    W0 = np.where(D <= -1, g[:, np.clip(D + Q, 0, Q - 1)], 0.0)
    w2 = np.stack([W0, W1], axis=2)  # [L, q, 2, i]
    w2 = np.ascontiguousarray(w2.transpose(1, 0, 2, 3)).astype(F8)

    x3 = x.reshape(L, NCH, Q)
    xq = np.zeros((Q, L, XC), F8)
    xq[:, :, 1:] = x3.transpose(2, 0, 1).astype(F8)

    in_maps = []
    for core in range(NCORES):
        sl = slice(core * LPC, (core + 1) * LPC)
        in_maps.append(
            {
                "xq": np.ascontiguousarray(xq[:, sl, :]),
                "w2": np.ascontiguousarray(w2[:, sl, :, :]),
            }
        )
    return in_maps


def kernel(x: np.ndarray, a: np.ndarray) -> np.ndarray:
    from concourse import bass_utils

    x = np.ascontiguousarray(x, dtype=np.float32)
    a = np.ascontiguousarray(a, dtype=np.float32)
    in_maps = _pack(x, a)

    nc = _get_bass()
    res = bass_utils.run_bass_kernel_spmd(
        nc,
        in_maps,
        core_ids=list(range(NCORES)),
        trace=bool(_cache.get("trace", False)),
        trace_cores=_cache.get("trace_cores"),
    )
    _cache["last_results"] = res

    y = np.empty((L, T), np.float32)
    for core in range(NCORES):
        c = res.results[core]["c"].astype(np.float32)  # [i, lane, t]
        sl = slice(core * LPC, (core + 1) * LPC)
        y[sl] = c.transpose(1, 2, 0).reshape(LPC, T)  # [lane, t, i]
    y += x.reshape(L, T)
    return y.reshape(B, C, T)
